# revision 10
# baseline (speedup 1.0000x reference)
"""Self-contained Trainium2 Bass kernel for nn_Attention_37125697306831.

Multi-head attention block: B=4, H=W=48 (N=2304), C=256, 8 heads, head_dim=32,
RoPE (rotate-half), softmax attention, separate Q/K/V projections (K without
bias), output projection with bias.

Sharding: 8 cores = (batch b in 0..3) x (query half in 0..1). Each core:
  - computes Q for its 1152 queries (all heads), K/V for all 2304 keys of its
    batch, attention + output projection for its 1152 query rows.
  - no collectives; output rows are disjoint across cores.

On-chip layouts:
  - xT [ci, n], qT/kT [c, n] (head dim on partitions), V natural [n, c].
  - scores computed transposed S.T[m keys, n queries] via row-packed K=32
    fp16 matmuls (tile_position), exp on ScalarE PSUM->SBUF (bottleneck
    engine), A@V as col-packed fp16 matmuls contracting over keys (K=128),
    softmax sums via ones-column matmuls, normalization via per-head K=1
    broadcast matmuls + DVE multiply, output projection consumes normalized
    out.T as lhsT giving y [n, co] for contiguous DMA out.

All matmul operands are fp16 (PE full rate; PSUM accumulation is fp32);
elementwise math (RoPE, exp, reciprocal, bias adds) stays fp32.
"""

import numpy as np
from contextlib import ExitStack

import concourse.bass as bass
import concourse.tile as tile
from concourse import bacc, mybir
from concourse.bass_utils import run_bass_kernel_spmd

F32 = mybir.dt.float32
F16 = mybir.dt.float16
AF = mybir.ActivationFunctionType

B, HH, WW, C = 4, 48, 48, 256
N = HH * WW            # 2304 keys per batch
NQ = N // 2            # 1152 queries per core
NH, HD, D2 = 8, 32, 16
NT = N // 128          # 18 key m-tiles
ROPE_BASE = 10000.0
SCALE = HD ** -0.5

QCH = [(0, 512), (512, 512), (1024, 128)]                       # query chunks
KCH = [(0, 512), (512, 512), (1024, 512), (1536, 512), (2048, 256)]
GROUPS = [(0, 3), (3, 3), (6, 2)]                               # (head0, size)

# DRAM input dtypes: fp16 for matmul operands, fp32 for DVE-side constants
IN_SPECS = [
    ("xT", [C, N], F16), ("xTq", [C, NQ], F16),
    ("wqT", [C, C], F16), ("wkT", [C, C], F16), ("wvT", [C, C], F16),
    ("woT", [C, C], F16),
    ("qb", [C, 1], F32), ("vb", [1, C], F16),
    ("ones", [128, 128], F16), ("RT", [128, 128], F16),
    ("bob", [128, C], F32),
    ("CTQ", [C, NQ], F32), ("STQ", [C, NQ], F32),
    ("CTK", [C, N], F32), ("STK", [C, N], F32),
]


def emit(tc, io, R=1):
    nc = tc.nc
    ctx = ExitStack()
    with ctx:
        consts = ctx.enter_context(tc.tile_pool(name="consts", bufs=1))
        sb = ctx.enter_context(tc.tile_pool(name="sb", bufs=1))
        tmp = ctx.enter_context(tc.tile_pool(name="tmp", bufs=3))
        ptpool = ctx.enter_context(tc.tile_pool(name="pt", bufs=3))
        outpool = ctx.enter_context(tc.tile_pool(name="outT", bufs=2))
        ypool = ctx.enter_context(tc.tile_pool(name="y", bufs=3))
        rpool = ctx.enter_context(tc.tile_pool(name="recip", bufs=2))
        # PSUM: scores 2x3 banks + av 1 + small 1 = 8 banks
        scp = ctx.enter_context(tc.tile_pool(name="scp", bufs=2, space="PSUM"))
        avp = ctx.enter_context(tc.tile_pool(name="avp", bufs=1, space="PSUM"))
        smp = ctx.enter_context(tc.tile_pool(name="smp", bufs=1, space="PSUM"))

        dtypes = {name: dt for name, _, dt in IN_SPECS}

        if R > 1:
            loop_ctx = tc.For_i(0, R, 1)
            loop_ctx.__enter__()

        def load(name, shape):
            t = consts.tile(shape, dtypes[name], tag=name)
            nc.sync.dma_start(t[:], io[name][:])
            return t

        # ---- constant loads ----------------------------------------------
        xT0 = consts.tile([128, N], F16, tag="xT0")
        nc.sync.dma_start(xT0[:], io["xT"][0:128, :])
        xT1 = consts.tile([128, N], F16, tag="xT1")
        nc.sync.dma_start(xT1[:], io["xT"][128:256, :])
        xTq0 = consts.tile([128, NQ], F16, tag="xTq0")
        nc.sync.dma_start(xTq0[:], io["xTq"][0:128, :])
        xTq1 = consts.tile([128, NQ], F16, tag="xTq1")
        nc.sync.dma_start(xTq1[:], io["xTq"][128:256, :])

        def load2(name, free):
            a = consts.tile([128, free], dtypes[name], tag=name + "0")
            nc.sync.dma_start(a[:], io[name][0:128, :])
            b_ = consts.tile([128, free], dtypes[name], tag=name + "1")
            nc.sync.dma_start(b_[:], io[name][128:256, :])
            return [a, b_]

        wq = load2("wqT", C)
        wk = load2("wkT", C)
        wv = load2("wvT", C)
        wo = load2("woT", C)
        qb = load2("qb", 1)
        vb = load("vb", [1, C])
        ones = load("ones", [128, 128])
        RT = load("RT", [128, 128])
        bob = load("bob", [128, C])
        ctq = load2("CTQ", NQ)
        stq = load2("STQ", NQ)
        ctk = load2("CTK", N)
        stk = load2("STK", N)

        # ---- phase 1: projections + RoPE ---------------------------------
        qT = [sb.tile([128, NQ], F16, tag=f"qT{i}", name=f"qT{i}") for i in range(2)]
        kT = [sb.tile([128, N], F16, tag=f"kT{i}", name=f"kT{i}") for i in range(2)]
        vsb = sb.tile([128, NT * C], F16, tag="v")

        def rope(dst, raw_sb, ct, st, off, cw):
            # partner = RT.T @ raw (signed rotate-half), then
            # dst = raw*ct + partner*st   (dst is fp16)
            pps = scp.tile([128, 3, 512], F32, tag="sc")
            nc.tensor.matmul(
                pps[:, 0, :cw], RT[:], raw_sb[:, 0:cw], start=True, stop=True)
            t1 = tmp.tile([128, 512], F32, tag="t1")
            nc.vector.tensor_mul(t1[:, 0:cw], raw_sb[:, 0:cw], ct[:, off:off + cw])
            t2 = tmp.tile([128, 512], F32, tag="t2")
            nc.vector.tensor_mul(t2[:, 0:cw], pps[:, 0, 0:cw], st[:, off:off + cw])
            nc.vector.tensor_add(dst[:, off:off + cw], t1[:, 0:cw], t2[:, 0:cw])

        for cg in range(2):
            # q projection (only our query columns) + bias + rope
            for off, cw in QCH:
                ps = scp.tile([128, 3, 512], F32, tag="sc")
                nc.tensor.matmul(ps[:, 0, :cw], wq[0][:, bass.ts(cg, 128)],
                                 xTq0[:, off:off + cw], start=True, stop=False)
                nc.tensor.matmul(ps[:, 0, :cw], wq[1][:, bass.ts(cg, 128)],
                                 xTq1[:, off:off + cw], start=False, stop=True)
                qraw = tmp.tile([128, 512], F16, tag="qraw")
                nc.vector.tensor_scalar_add(qraw[:, 0:cw], ps[:, 0, 0:cw], qb[cg][:])
                rope(qT[cg], qraw, ctq[cg], stq[cg], off, cw)
            # k projection (all keys), no bias, + rope
            for off, cw in KCH:
                ps = scp.tile([128, 3, 512], F32, tag="sc")
                nc.tensor.matmul(ps[:, 0, :cw], wk[0][:, bass.ts(cg, 128)],
                                 xT0[:, off:off + cw], start=True, stop=False)
                nc.tensor.matmul(ps[:, 0, :cw], wk[1][:, bass.ts(cg, 128)],
                                 xT1[:, off:off + cw], start=False, stop=True)
                kraw = tmp.tile([128, 512], F16, tag="kraw")
                nc.vector.tensor_copy(kraw[:, 0:cw], ps[:, 0, 0:cw])
                rope(kT[cg], kraw, ctk[cg], stk[cg], off, cw)

        # v projection (natural layout [n, c]) + bias via K=1 matmul
        for t in range(NT):
            ps = scp.tile([128, 3, 512], F32, tag="sc")
            nc.tensor.matmul(ps[:, 0, :C], xT0[:, bass.ts(t, 128)], wv[0][:],
                             start=True, stop=False)
            nc.tensor.matmul(ps[:, 0, :C], xT1[:, bass.ts(t, 128)], wv[1][:],
                             start=False, stop=False)
            nc.tensor.matmul(ps[:, 0, :C], ones[0:1, :], vb[:],
                             start=False, stop=True)
            nc.vector.tensor_copy(vsb[:, bass.ts(t, C)], ps[:, 0, 0:C])

        # ---- phase 2+3: attention + output projection --------------------
        for qoff, cw in QCH:
            oT0 = outpool.tile([128, 512], F16, tag="o0")
            oT1 = outpool.tile([128, 512], F16, tag="o1")
            for hs, gsz in GROUPS:
                av = avp.tile([128, 512], F32, tag="av")
                sm = smp.tile([128, 512], F32, tag="sm")
                for t in range(NT):
                    sc = scp.tile([128, 3, 512], F32, tag="sc")
                    for gi in range(gsz):
                        h = hs + gi
                        cg, hh = h // 4, h % 4
                        nc.tensor.matmul(
                            sc[:, gi, :cw],
                            kT[cg][bass.ts(hh, 32), bass.ts(t, 128)],
                            qT[cg][bass.ts(hh, 32), qoff:qoff + cw],
                            start=True, stop=True, tile_position=(32 * hh, 0))
                    pt = ptpool.tile([128, 3, 512], F16, tag="pt")
                    nc.scalar.activation(pt[:, 0:gsz, 0:cw], sc[:, 0:gsz, 0:cw],
                                         AF.Exp)
                    for gi in range(gsz):
                        h = hs + gi
                        nc.tensor.matmul(
                            av[bass.ts(gi, 32), 0:cw],
                            vsb[:, t * C + 32 * h: t * C + 32 * h + 32],
                            pt[:, gi, 0:cw],
                            start=(t == 0), stop=(t == NT - 1),
                            tile_position=(0, 32 * gi),
                            skip_group_check=True)
                        nc.tensor.matmul(
                            sm[32 * gi: 32 * gi + 1, 0:cw],
                            ones[:, 0:1],
                            pt[:, gi, 0:cw],
                            start=(t == 0), stop=(t == NT - 1),
                            tile_position=(0, 32 * gi),
                            skip_group_check=True)
                # finalize group: recip of sums (fp32), fp16 row for the
                # broadcast matmul, then normalize
                rsb32 = rpool.tile([128, 512], F32, tag="rs32")
                rsb = rpool.tile([128, 512], F16, tag="rs")
                for gi in range(gsz):
                    nc.vector.reciprocal(rsb32[32 * gi:32 * gi + 1, 0:cw],
                                         sm[32 * gi:32 * gi + 1, 0:cw])
                    nc.vector.tensor_copy(rsb[32 * gi:32 * gi + 1, 0:cw],
                                          rsb32[32 * gi:32 * gi + 1, 0:cw])
                rf = smp.tile([128, 512], F32, tag="sm")
                for gi in range(gsz):
                    nc.tensor.matmul(rf[bass.ts(gi, 32), 0:cw],
                                     ones[32 * gi:32 * gi + 1, 0:32],
                                     rsb[32 * gi:32 * gi + 1, 0:cw],
                                     start=True, stop=True,
                                     tile_position=(32 * gi, 32 * gi),
                                     skip_group_check=True)
                # DVE can read only one PSUM operand: stage rf in SBUF
                rfsb = rpool.tile([128, 512], F32, tag="rfsb")
                nc.vector.tensor_copy(rfsb[0:32 * gsz, 0:cw], rf[0:32 * gsz, 0:cw])
                # write normalized out.T rows [32*hs, 32*(hs+gsz)) across the
                # two 128-row tiles
                c0, c1 = 32 * hs, 32 * (hs + gsz)
                pieces = []
                if c0 < 128:
                    pieces.append((c0, min(c1, 128), oT0, c0))
                if c1 > 128:
                    pieces.append((max(c0, 128), c1, oT1, max(c0, 128) - 128))
                for (a, b_, dst, d0) in pieces:
                    # 32-partition blocks keep every AP in a legal
                    # (32-aligned start, <=32 count) partition window
                    for blk in range(0, b_ - a, 32):
                        s0 = a - c0 + blk
                        nc.vector.tensor_mul(dst[d0 + blk:d0 + blk + 32, 0:cw],
                                             av[s0:s0 + 32, 0:cw],
                                             rfsb[s0:s0 + 32, 0:cw])
            # output projection for this chunk
            for s in range(cw // 128):
                yps = scp.tile([128, 3, 512], F32, tag="sc")
                nc.tensor.matmul(yps[:, 0, :C], oT0[:, bass.ts(s, 128)],
                                 wo[0][:], start=True, stop=False)
                nc.tensor.matmul(yps[:, 0, :C], oT1[:, bass.ts(s, 128)],
                                 wo[1][:], start=False, stop=True)
                ysb = ypool.tile([128, C], F32, tag="y")
                nc.vector.tensor_add(ysb[:], yps[:, 0, 0:C], bob[:])
                nc.sync.dma_start(io["y"][qoff + 128 * s: qoff + 128 * (s + 1), :],
                                  ysb[:])

        if R > 1:
            loop_ctx.__exit__(None, None, None)


def build_nc(R=1):
    nc = bacc.Bacc("TRN2", target_bir_lowering=False, debug=False,
                   enable_asserts=True, num_devices=8)
    io = {}
    for name, shape, dt in IN_SPECS:
        io[name] = nc.dram_tensor(name, shape, dt, kind="ExternalInput").ap()
    io["y"] = nc.dram_tensor("y", [NQ, C], F32, kind="ExternalOutput").ap()

    with tile.TileContext(nc) as tc:
        emit(tc, io, R=R)
    nc.compile()
    return nc


def host_inputs(x, Wq, q_bias, Wk, Wv, v_bias, Wo, bo):
    """Build the per-core input maps (host-side sharding + layout prep)."""
    xf = np.ascontiguousarray(x.reshape(B, N, C))

    inv_freq = 1.0 / (ROPE_BASE ** (np.arange(0, HD, 2, dtype=np.float64) / HD))
    pos = np.arange(N, dtype=np.float64)
    ang = pos[:, None] * inv_freq[None, :]          # [N, 16]
    cos_t, sin_t = np.cos(ang), np.sin(ang)         # [N, 16]
    # channel c -> within-head index jj = c % 32, freq f = jj % 16
    jj = np.arange(C) % HD
    f = jj % D2
    CT = cos_t[:, f].T.astype(np.float32)           # [C, N]
    ST = sin_t[:, f].T.astype(np.float32)           # [C, N]

    RT = np.zeros((128, 128), dtype=np.float16)
    for p in range(128):
        j = p % 32
        if j < D2:
            RT[p + D2, p] = -1.0                    # partner[p] = -q[p+16]
        else:
            RT[p - D2, p] = 1.0                     # partner[p] = +q[p-16]

    f16 = lambda a: np.ascontiguousarray(a, dtype=np.float16)
    f32 = lambda a: np.ascontiguousarray(a, dtype=np.float32)

    common = {
        "wqT": f16(Wq.T), "wkT": f16(Wk.T), "wvT": f16(Wv.T), "woT": f16(Wo.T),
        "qb": f32(q_bias[:, None]),
        "vb": f16(v_bias[None, :]),
        "ones": np.ones((128, 128), dtype=np.float16),
        "RT": RT,
        "bob": f32(np.broadcast_to(bo, (128, C))),
        "CTK": CT, "STK": ST,
    }
    in_maps = []
    for core in range(8):
        b, qhalf = core // 2, core % 2
        qoff = qhalf * NQ
        xT = xf[b].T
        m = dict(common)
        m["xT"] = f16(xT)
        m["xTq"] = f16(xT[:, qoff:qoff + NQ])
        m["CTQ"] = f32(CT[:, qoff:qoff + NQ] * SCALE)
        m["STQ"] = f32(ST[:, qoff:qoff + NQ] * SCALE)
        in_maps.append(m)
    return in_maps


_NC_CACHE = {}


def get_nc(R=1):
    if R not in _NC_CACHE:
        _NC_CACHE[R] = build_nc(R)
    return _NC_CACHE[R]


def kernel(**inputs):
    inputs = {k: np.asarray(v, dtype=np.float32) for k, v in inputs.items()}
    in_maps = host_inputs(**inputs)
    nc = get_nc()
    res = run_bass_kernel_spmd(nc, in_maps, core_ids=list(range(8)))
    out = np.empty((B, N, C), dtype=np.float32)
    for core in range(8):
        b, qhalf = core // 2, core % 2
        qoff = qhalf * NQ
        out[b, qoff:qoff + NQ, :] = res.results[core]["y"]
    return out.reshape(B, HH, WW, C)
